# revision 1
# baseline (speedup 1.0000x reference)
"""CAGroup3DHead kernel for 8 Trainium2 NeuronCores.

Strategy (data-parallel over voxels, per the sharding hint):
  - Host: integer index work (sorted-key neighbor lookup identical to the
    reference), weight fusion (BN folded into weights, ELU+1 bias shifts,
    per-class reg expansion folded into a [C,108] weight), and sharding
    marshaling (transpose to channel-major, bf16 cast, per-core slices).
    The 3x3x3 sparse conv collapses to a gather: the (0,0,0) tap always
    hits, so conv_in = feats[rep]; the rare other-tap hits are folded into
    conv_in via W_k @ W_13^{-1} so the device conv is one dense matmul.
  - Device (identical SPMD program on 8 cores): per 512-voxel tile, 9
    bf16 matmuls in channel-major layout; ELU+1 computed exactly as
    min(relu(y)+1, exp(y)) with exp/relu on ScalarE and the min on
    VectorE; masked per-class outputs via an expansion matmul; outputs
    stored transposed and re-transposed on the host during unsharding.
"""

import numpy as np
import ml_dtypes

import concourse.bass as bass
import concourse.bacc as bacc
import concourse.tile as tile
from concourse import mybir
from concourse.bass_utils import run_bass_kernel_spmd

BF16 = ml_dtypes.bfloat16

N_VOX = 100000
C = 128
N_CLS = 18
N_REG = 6
VS = 0.04
THR = 0.15
HASH_D = 260
N_CORES = 8
PER_CORE = N_VOX // N_CORES          # 12500
T = 512                              # voxels per macro-tile
N_TILES = 25
PAD = T * N_TILES                    # 12800 padded voxels per core
LOGIT_THR = float(np.log(THR / (1.0 - THR)))   # -1.734601..

# device outT (f32): rows 0:18 sem, 18:21 voff, 21:24 voted, 24:25 cen
# device outB (bf16): rows 0:18 cls, 18:126 regpc
DEV_ROWS = 25
OUT_ROWS = 151

F32 = mybir.dt.float32
BF = mybir.dt.bfloat16
AOp = mybir.AluOpType
Act = mybir.ActivationFunctionType


def _build_program(n_tiles):
    nc = bacc.Bacc(trn_type="TRN2")

    pad = T * n_tiles
    xT_d = nc.dram_tensor("xT", [C, pad], BF, kind="ExternalInput")
    gT_d = nc.dram_tensor("gT", [C, pad], BF, kind="ExternalInput")
    cvs_d = nc.dram_tensor("cvs", [3, pad], F32, kind="ExternalInput")
    # bf16 weights packed column-wise (one DMA): w1 0:128, w2 128:256,
    # wc 256:384, semw 384:416, w3 416:448, wcen 448:480, wcls(half) 480:512,
    # wreg 512:620, e2s 620:728 (rows 0:18), clsb-half row 728:760 (row 0)
    wb_d = nc.dram_tensor("wb", [C, 760], BF, kind="ExternalInput")
    # per-partition scalars [128, 8] f32: col0 b1, col1 b2, col2 bc,
    # col3 bias96 (rows 0:96), col4 clsb (rows 0:18), col5 b108 (rows 0:108),
    # col6 min bound (rows 32:35), col7 max bound (rows 32:35)
    sc_d = nc.dram_tensor("sc", [C, 8], F32, kind="ExternalInput")
    out_d = nc.dram_tensor("outT", [DEV_ROWS, pad], F32, kind="ExternalOutput")
    outb_d = nc.dram_tensor("outB", [126, pad], BF, kind="ExternalOutput")

    with tile.TileContext(nc) as tc:
        with (
            tc.tile_pool(name="wpool", bufs=1) as wpool,
            tc.tile_pool(name="loads", bufs=4) as loads,
            tc.tile_pool(name="work", bufs=4) as work,
            tc.tile_pool(name="outs", bufs=4) as outs,
            tc.tile_pool(name="psum", bufs=1, space=bass.MemorySpace.PSUM) as pp,
            tc.tile_pool(name="psum2", bufs=1, space=bass.MemorySpace.PSUM) as pp2,
            tc.tile_pool(name="psum3", bufs=2, space=bass.MemorySpace.PSUM) as pp3,
        ):
            wb = wpool.tile([C, 760], BF)
            sc = wpool.tile([C, 8], F32)
            nc.sync.dma_start(wb[:], wb_d[:])
            nc.sync.dma_start(sc[:], sc_d[:])
            w1 = wb[:, 0:128]
            w2 = wb[:, 128:256]
            wc = wb[:, 256:384]
            semw = wb[:, 384:416]
            w3 = wb[:, 416:448]
            wcen = wb[:, 448:480]
            wcls = wb[:, 480:512]
            wreg = wb[:, 512:620]
            e2s = wb[0:N_CLS, 620:728]
            clsbw = wb[0:1, 728:760]
            b1 = sc[:, 0:1]
            b2 = sc[:, 1:2]
            bc = sc[:, 2:3]
            bias96 = sc[0:96, 3:4]
            b108 = sc[0:108, 5:6]
            minb = sc[32:35, 6:7]
            maxb = sc[32:35, 7:8]
            sthr = sc[0:N_CLS, 4:5]
            ones = wpool.tile([1, T], BF)
            nc.gpsimd.memset(ones[:], 1.0)

            for i in range(n_tiles):
                cs = bass.ts(i, T)
                xT = loads.tile([C, T], BF)
                gT = loads.tile([C, T], BF)
                cvs = loads.tile([35, T], F32)
                nc.sync.dma_start(xT[:], xT_d[:, cs])
                nc.sync.dma_start(gT[:], gT_d[:, cs])
                nc.sync.dma_start(cvs[32:35, :], cvs_d[:, cs])

                # ---- MLP layer 1: f1 = ELU(x@W1 + b1) + 1 ----
                p_y1 = pp3.tile([C, T], F32, tag="p_y1")
                nc.tensor.matmul(p_y1[:], w1, xT[:], start=True, stop=True)
                e1 = work.tile([C, T], BF, tag="e1")
                nc.scalar.activation(e1[:], p_y1[:], Act.Exp, bias=b1)
                r1 = work.tile([C, T], BF, tag="r1")
                nc.scalar.activation(r1[:], p_y1[:], Act.Relu, bias=b1)
                f1 = work.tile([C, T], BF, tag="f1")
                nc.vector.scalar_tensor_tensor(
                    f1[:], r1[:], 1.0, e1[:], AOp.add, AOp.min)

                # ---- conv: fo = ELU(g@Wc + bc) + 1 ----
                p_yc = pp2.tile([C, T], F32, tag="p_yc")
                nc.tensor.matmul(p_yc[:], wc, gT[:], start=True, stop=True)
                ec = work.tile([C, T], BF, tag="ec")
                nc.scalar.activation(ec[:], p_yc[:], Act.Exp, bias=bc)
                rc = work.tile([C, T], BF, tag="rc")
                nc.scalar.activation(rc[:], p_yc[:], Act.Relu, bias=bc)
                fo = work.tile([C, T], BF, tag="fo")
                nc.vector.scalar_tensor_tensor(
                    fo[:], rc[:], 1.0, ec[:], AOp.add, AOp.min)

                # ---- MLP layer 2: f2 = ELU(f1@W2 + b2') + 1 ----
                p_y2 = pp.tile([C, T], F32, tag="p_y2")
                nc.tensor.matmul(p_y2[:], w2, f1[:], start=True, stop=True)
                e2 = work.tile([C, T], BF, tag="e2")
                nc.scalar.activation(e2[:], p_y2[:], Act.Exp, bias=b2)
                r2 = work.tile([C, T], BF, tag="r2")
                nc.scalar.activation(r2[:], p_y2[:], Act.Relu, bias=b2)
                f2 = work.tile([C, T], BF, tag="f2")
                nc.vector.scalar_tensor_tensor(
                    f2[:], r2[:], 1.0, e2[:], AOp.add, AOp.min)

                # ---- small heads, col-tiled into one PSUM bank ----
                # G0 rows 0:32 sem <- x; G1 32:64 voff <- f2; G2 64:96 cen <- fo
                p_s = pp.tile([C, T], F32, tag="p_s")
                nc.tensor.matmul(p_s[0:32, :], semw, xT[:],
                                 start=True, stop=True, tile_position=(0, 0))
                nc.tensor.matmul(p_s[32:64, :], w3, f2[:],
                                 start=True, stop=True, tile_position=(0, 32))
                nc.tensor.matmul(p_s[64:96, :], wcen, fo[:],
                                 start=True, stop=True, tile_position=(0, 64))

                # biases for all small rows in one op (junk rows harmless)
                so = outs.tile([96, T], F32, tag="so")
                nc.vector.tensor_scalar(so[:], p_s[0:96, :], bias96, None, AOp.add)

                # s = sign(sem - logit(thr)) in {-1,0,1}; mask = (s+1)/2
                s_t = outs.tile([N_CLS, T], BF, tag="s_t")
                nc.scalar.activation(s_t[:], p_s[0:N_CLS, :], Act.Sign,
                                     bias=sthr)

                # voted = clip(voff + coords*VS) on GpSimd (tensor_tensor only)
                v1 = outs.tile([35, T], F32, tag="v1")
                nc.gpsimd.tensor_tensor(v1[32:35, :], so[32:35, :],
                                        cvs[32:35, :], AOp.add)
                voted = outs.tile([35, T], F32, tag="voted")
                nc.vector.tensor_scalar(voted[32:35, :], v1[32:35, :],
                                        minb, maxb, AOp.max, AOp.min)

                # cls = (s+1) * (cls_pre + clsb)/2  (weights pre-halved)
                p_cls = pp.tile([32, T], F32, tag="p_cls")
                nc.tensor.matmul(p_cls[:], wcls, fo[:], start=True, stop=False)
                nc.tensor.matmul(p_cls[:], clsbw, ones[:], start=False, stop=True)
                cls_o = outs.tile([N_CLS, T], BF, tag="cls_o")
                nc.vector.scalar_tensor_tensor(
                    cls_o[:], s_t[:], 1.0, p_cls[0:N_CLS, :], AOp.add, AOp.mult)

                # ---- per-class reg expansion ----
                p_r = pp.tile([108, T], F32, tag="p_r")
                nc.tensor.matmul(p_r[:], wreg, fo[:], start=True, stop=True)
                p_m = pp.tile([108, T], F32, tag="p_m")
                nc.tensor.matmul(p_m[:], e2s, s_t[:], start=True, stop=True)
                mexp_s = work.tile([108, T], F32, tag="mexp_s")
                nc.scalar.activation(mexp_s[:], p_m[:], Act.Copy, bias=0.5,
                                     scale=0.5)
                regpc = outs.tile([108, T], BF, tag="regpc")
                nc.vector.scalar_tensor_tensor(
                    regpc[:], p_r[:], b108, mexp_s[:], AOp.add, AOp.mult)

                # ---- stores (4 DMAs) ----
                nc.sync.dma_start(out_d[0:18, cs], so[0:18, :])
                nc.sync.dma_start(out_d[18:21, cs], so[32:35, :])
                nc.sync.dma_start(out_d[24:25, cs], so[64:65, :])
                nc.sync.dma_start(out_d[21:24, cs], voted[32:35, :])
                nc.sync.dma_start(outb_d[0:18, cs], cls_o[:])
                nc.sync.dma_start(outb_d[18:126, cs], regpc[:])

    nc.finalize()
    return nc


def _host_prep(feats, coords_xyz, batch_idx,
               off_w1, off_g1, off_b1, off_w2, off_g2, off_b2, off_w3,
               fo_w, fo_g, fo_b, sem_w, sem_b, cen_w, cls_w, cls_b, reg_w,
               scales):
    f64 = np.float64
    N = feats.shape[0]

    # ---- neighbor lookup (identical to reference's sorted-key search) ----
    c1 = coords_xyz.astype(np.int64) + 1
    key = ((batch_idx.astype(np.int64) * HASH_D + c1[:, 0]) * HASH_D
           + c1[:, 1]) * HASH_D + c1[:, 2]
    order = np.argsort(key, kind="stable")
    skey = key[order]
    pos = np.searchsorted(skey, key)
    rep = order[pos]                      # first voxel with same key

    # ---- fused weights (BN folded; ELU+1 handled via bias shifts) ----
    W1 = off_w1.astype(f64) * off_g1.astype(f64)[None, :]
    b1 = off_b1.astype(f64)
    W2 = off_w2.astype(f64) * off_g2.astype(f64)[None, :]
    b2 = off_b2.astype(f64) - W2.sum(0)
    W3 = off_w3.astype(f64)
    c3 = -W3.sum(0)
    Wc = fo_w[13].astype(f64) * fo_g.astype(f64)[None, :]
    bc = fo_b.astype(f64)

    # ---- conv input: gather + fold rare non-center taps via Wc13^-1 ----
    G = feats.astype(f64)[rep]
    Winv = np.linalg.inv(fo_w[13].astype(f64))
    k = 0
    for dx in (-1, 0, 1):
        for dy in (-1, 0, 1):
            for dz in (-1, 0, 1):
                if (dx, dy, dz) != (0, 0, 0):
                    nk = key + (dx * HASH_D + dy) * HASH_D + dz
                    p = np.clip(np.searchsorted(skey, nk), 0, N - 1)
                    hit = skey[p] == nk
                    if hit.any():
                        dst = np.nonzero(hit)[0]
                        src = order[p[hit]]
                        A = fo_w[k].astype(f64) @ Winv
                        np.add.at(G, dst, feats.astype(f64)[src] @ A)
                k += 1

    # ---- per-class reg expansion folded into [C,108] weight ----
    sc64 = scales.astype(f64)
    Wreg = (reg_w.astype(f64)[:, None, :] * sc64[None, :, None]).reshape(C, 108)
    b108 = (-reg_w.astype(f64).sum(0)[None, :] * sc64[:, None]).reshape(108)
    E2s = np.zeros((N_CLS, 108), np.float32)
    for c in range(N_CLS):
        E2s[c, N_REG * c:N_REG * (c + 1)] = 1.0

    # ---- per-partition scalar pack ----
    bias96 = np.zeros(96, f64)
    bias96[0:18] = sem_b.astype(f64)
    bias96[32:35] = c3
    bias96[64] = -cen_w.astype(f64).sum(0)[0]
    mx = (coords_xyz.max(0) + 1).astype(f64) * VS
    mn = (coords_xyz.min(0) - 1).astype(f64) * VS
    sc = np.zeros((C, 8), np.float32)
    sc[:, 0] = b1
    sc[:, 1] = b2
    sc[:, 2] = bc
    sc[0:96, 3] = bias96
    sc[0:N_CLS, 4] = sem_b.astype(f64) - LOGIT_THR
    sc[0:108, 5] = b108
    sc[32:35, 6] = mn
    sc[32:35, 7] = mx

    # ---- weights blob ----
    wb = np.zeros((C, 760), BF16)
    wb[:, 0:128] = W1.astype(BF16)
    wb[:, 128:256] = W2.astype(BF16)
    wb[:, 256:384] = Wc.astype(BF16)
    wb[:, 384:402] = sem_w.astype(f64).astype(BF16)
    wb[:, 416:419] = W3.astype(BF16)
    wb[:, 448:449] = cen_w.astype(f64).astype(BF16)
    wb[:, 480:498] = (cls_w.astype(f64) * 0.5).astype(BF16)
    wb[0, 728:746] = ((cls_b.astype(f64) - cls_w.astype(f64).sum(0)) * 0.5
                      ).astype(BF16)
    wb[:, 512:620] = Wreg.astype(BF16)
    wb[0:N_CLS, 620:728] = E2s.astype(BF16)

    # ---- transposed, padded, channel-major activations ----
    xT = np.zeros((C, N_CORES * PAD), BF16)
    gT = np.zeros((C, N_CORES * PAD), BF16)
    cvs = np.zeros((3, N_CORES * PAD), np.float32)
    fT = np.ascontiguousarray(feats.T)
    gTf = np.ascontiguousarray(G.astype(np.float32).T)
    cT = coords_xyz.T.astype(np.float32) * VS
    for c in range(N_CORES):
        s, e = c * PER_CORE, (c + 1) * PER_CORE
        xT[:, c * PAD:c * PAD + PER_CORE] = fT[:, s:e].astype(BF16)
        gT[:, c * PAD:c * PAD + PER_CORE] = gTf[:, s:e].astype(BF16)
        cvs[:, c * PAD:c * PAD + PER_CORE] = cT[:, s:e]

    wts = {"wb": wb, "sc": sc}
    in_maps = []
    for c in range(N_CORES):
        m = dict(wts)
        m["xT"] = np.ascontiguousarray(xT[:, c * PAD:(c + 1) * PAD])
        m["gT"] = np.ascontiguousarray(gT[:, c * PAD:(c + 1) * PAD])
        m["cvs"] = np.ascontiguousarray(cvs[:, c * PAD:(c + 1) * PAD])
        in_maps.append(m)
    return in_maps


_CACHED = {}


def _untranspose(outT, outB, n):
    """Map device outputs to reference layout [n, 151]."""
    o = np.empty((n, OUT_ROWS), np.float32)
    o[:, 0:25] = outT[:, :n].T
    o[:, 25:151] = outB[:, :n].astype(np.float32).T
    return o


def kernel(**inputs):
    inputs = {k: np.asarray(v) for k, v in inputs.items()}
    in_maps = _host_prep(**inputs)
    if "nc" not in _CACHED:
        _CACHED["nc"] = _build_program(N_TILES)
    nc = _CACHED["nc"]
    res = run_bass_kernel_spmd(nc, in_maps, core_ids=list(range(N_CORES)))
    out = np.empty((N_VOX, OUT_ROWS), np.float32)
    for c in range(N_CORES):
        out[c * PER_CORE:(c + 1) * PER_CORE] = _untranspose(
            res.results[c]["outT"], res.results[c]["outB"], PER_CORE)
    return out



# revision 5
# speedup vs baseline: 1.0564x; 1.0564x over previous
"""CAGroup3DHead kernel for 8 Trainium2 NeuronCores.

Strategy (data-parallel over voxels, per the sharding hint):
  - Host: integer index work (sorted-key neighbor lookup identical to the
    reference), weight fusion (BN folded into weights, ELU+1 bias shifts,
    per-class reg expansion folded into a [C,108] j-major weight), and
    sharding marshaling (transpose to channel-major, bf16 cast, per-core
    slices).  The 3x3x3 sparse conv collapses to a gather: the (0,0,0) tap
    always hits, so conv_in = feats[rep]; the rare other-tap hits are folded
    into conv_in via W_k @ W_13^{-1} so the device conv is one dense matmul.
  - Device (identical SPMD program on 8 cores): inputs and outputs are
    SBUF-resident for the whole kernel (a handful of large DMAs instead of
    per-tile ones).  Work proceeds in 1024-column supertiles: per supertile
    the three 128x128 matmul chains write 2-bank PSUM tiles (biases +1
    accumulated via a ones-row matmul), each ELU+1 is computed exactly as
    min(max(y+b+1, 1), exp(y+b)) with one scalar Exp and one vector STT;
    per-class semantic gating uses a [128,127] j-major sem weight, one
    is_gt vector op for the 0/1 mask, and one STT (p_foh + biasF) * mask
    producing cen|cls|regpc in a single [127,T] stream.  Outputs are bf16;
    the host re-transposes and permutes rows during unsharding.
"""

import numpy as np
import ml_dtypes

import concourse.bass as bass
import concourse.bacc as bacc
import concourse.tile as tile
from concourse import mybir
from concourse.bass_utils import run_bass_kernel_spmd

BF16 = ml_dtypes.bfloat16

N_VOX = 100000
C = 128
N_CLS = 18
N_REG = 6
VS = 0.04
THR = 0.15
HASH_D = 260
N_CORES = 8
PER_CORE = N_VOX // N_CORES          # 12500
ST = 1024                            # supertile columns
N_ST = 13                            # 12 full + tail of 212
LOGIT_THR = float(np.log(THR / (1.0 - THR)))   # -1.734601..

# fohF/maskF/outB row layout: [cls 0:18 | cen 18 | regJ 19:127] with
# regJ row 19 + j*18 + c  ==  scale_c * reg_j  (j-major)
FOH = 127
OUT_ROWS = 151

F32 = mybir.dt.float32
BF = mybir.dt.bfloat16
AOp = mybir.AluOpType
Act = mybir.ActivationFunctionType


def _build_program():
    nc = bacc.Bacc(trn_type="TRN2")

    xT_d = nc.dram_tensor("xT", [C, PER_CORE], BF, kind="ExternalInput")
    gT_d = nc.dram_tensor("gT", [C, PER_CORE], BF, kind="ExternalInput")
    cvs_d = nc.dram_tensor("cvs", [3, PER_CORE], BF, kind="ExternalInput")
    # bf16 weights packed column-wise: w1 0:128, wc 128:256, w2 256:384,
    # semJF 384:511, fohF 511:638, w3 638:641; row 0 extra: b1p1 641:769,
    # bcp1 769:897, b2p1 897:1025
    wb_d = nc.dram_tensor("wb", [C, 1025], BF, kind="ExternalInput")
    # f32 per-partition scalars [128, 7]: col0 semb (rows 0:18),
    # col1 thrF (0:127), col2 biasF (0:127), col3 c3 (0:3),
    # col4 mn (0:3), col5 mx (0:3), col6 -1.0 (all rows)
    sc_d = nc.dram_tensor("sc", [C, 7], F32, kind="ExternalInput")
    outS_d = nc.dram_tensor("outS", [N_CLS, PER_CORE], BF, kind="ExternalOutput")
    outV_d = nc.dram_tensor("outV", [3, PER_CORE], BF, kind="ExternalOutput")
    outW_d = nc.dram_tensor("outW", [3, PER_CORE], BF, kind="ExternalOutput")
    outB_d = nc.dram_tensor("outB", [FOH, PER_CORE], BF, kind="ExternalOutput")

    # input/output DMA chunk boundaries (supertile-aligned)
    chunks = [(0, 3072), (3072, 6144), (6144, 9216), (9216, PER_CORE)]

    with tile.TileContext(nc) as tc:
        with (
            tc.tile_pool(name="res", bufs=1) as res,
            tc.tile_pool(name="epool", bufs=4) as epool,
            tc.tile_pool(name="fpool", bufs=4) as fpool,
            tc.tile_pool(name="mpool", bufs=3) as mpool,
            tc.tile_pool(name="vpool", bufs=3) as vpool,
            tc.tile_pool(name="pbig", bufs=2, space=bass.MemorySpace.PSUM) as pbig,
            tc.tile_pool(name="phead", bufs=2, space=bass.MemorySpace.PSUM) as ph,
        ):
            wb = res.tile([C, 1025], BF)
            sc = res.tile([C, 7], F32)
            nc.sync.dma_start(wb[:], wb_d[:])
            nc.sync.dma_start(sc[:], sc_d[:])
            w1 = wb[:, 0:128]
            wc = wb[:, 128:256]
            w2 = wb[:, 256:384]
            semJF = wb[:, 384:511]
            fohF = wb[:, 511:638]
            w3 = wb[:, 638:641]
            b1p1 = wb[0:1, 641:769]
            bcp1 = wb[0:1, 769:897]
            b2p1 = wb[0:1, 897:1025]
            semb = sc[0:N_CLS, 0:1]
            thrF = sc[0:FOH, 1:2]
            biasF = sc[0:FOH, 2:3]
            c3 = sc[0:3, 3:4]
            mn3 = sc[0:3, 4:5]
            mx3 = sc[0:3, 5:6]
            neg1 = sc[:, 6:7]

            ones = res.tile([1, 512], BF)
            nc.gpsimd.memset(ones[:], 1.0)

            # SBUF-resident inputs and outputs (small tensors streamed)
            xT = res.tile([C, PER_CORE], BF)
            gT = res.tile([C, PER_CORE], BF)
            outS = res.tile([N_CLS, PER_CORE], BF)
            outB = res.tile([FOH, PER_CORE], BF)
            for a, b in chunks:
                nc.sync.dma_start(xT[:, a:b], xT_d[:, a:b])
                nc.sync.dma_start(gT[:, a:b], gT_d[:, a:b])

            def big_mm(p, w, rhs_sb, cs, L, brow):
                """p[:, :L] = w.T @ rhs[:, cs:cs+L] + brow (ones trick)."""
                for a in range(0, L, 512):
                    e = min(a + 512, L)
                    nc.tensor.matmul(p[:, a:e], w, rhs_sb[:, cs + a:cs + e],
                                     start=True, stop=False)
                    nc.tensor.matmul(p[:, a:e], brow, ones[:, 0:e - a],
                                     start=False, stop=True)

            def head_mm(p, w, rhs_sb, cs, L, rows):
                for a in range(0, L, 512):
                    e = min(a + 512, L)
                    nc.tensor.matmul(p[0:rows, a:e], w,
                                     rhs_sb[:, cs + a:cs + e],
                                     start=True, stop=True)

            for k in range(N_ST):
                cs = k * ST
                L = min(ST, PER_CORE - cs)

                # ---- MLP layer 1: f1 = ELU(x@W1 + b1) + 1 ----
                p1 = pbig.tile([C, ST], F32, tag="p")
                big_mm(p1, w1, xT, cs, L, b1p1)
                e1 = epool.tile([C, ST], BF, tag="e")
                nc.scalar.activation(e1[:, :L], p1[:, :L], Act.Exp, bias=neg1)
                f1 = fpool.tile([C, ST], BF, tag="f")
                nc.vector.scalar_tensor_tensor(
                    f1[:, :L], p1[:, :L], 1.0, e1[:, :L], AOp.max, AOp.min)

                # ---- conv: fo = ELU(g@Wc + bc) + 1 ----
                pc = pbig.tile([C, ST], F32, tag="p")
                big_mm(pc, wc, gT, cs, L, bcp1)
                ec = epool.tile([C, ST], BF, tag="e")
                nc.scalar.activation(ec[:, :L], pc[:, :L], Act.Exp, bias=neg1)
                fo = fpool.tile([C, ST], BF, tag="f")
                nc.vector.scalar_tensor_tensor(
                    fo[:, :L], pc[:, :L], 1.0, ec[:, :L], AOp.max, AOp.min)

                # ---- MLP layer 2: f2 = ELU(f1@W2 + b2') + 1 ----
                p2 = pbig.tile([C, ST], F32, tag="p")
                for a in range(0, L, 512):
                    e = min(a + 512, L)
                    nc.tensor.matmul(p2[:, a:e], w2, f1[:, a:e],
                                     start=True, stop=False)
                    nc.tensor.matmul(p2[:, a:e], b2p1, ones[:, 0:e - a],
                                     start=False, stop=True)
                e2 = epool.tile([C, ST], BF, tag="e")
                nc.scalar.activation(e2[:, :L], p2[:, :L], Act.Exp, bias=neg1)
                f2 = fpool.tile([C, ST], BF, tag="f")
                nc.vector.scalar_tensor_tensor(
                    f2[:, :L], p2[:, :L], 1.0, e2[:, :L], AOp.max, AOp.min)

                # ---- sem logits (j-major expanded) + mask + sem output ----
                psem = ph.tile([FOH, ST], F32, tag="ph")
                head_mm(psem, semJF, xT, cs, L, FOH)
                maskF = mpool.tile([FOH, ST], BF, tag="maskF")
                nc.vector.tensor_scalar(maskF[:, :L], psem[:, :L], thrF, None,
                                        AOp.is_gt)
                nc.scalar.activation(outS[:, cs:cs + L], psem[0:N_CLS, :L],
                                     Act.Identity, bias=semb)

                # ---- masked heads: (foh + biasF) * mask -> cen|cls|regpc ----
                pfoh = ph.tile([FOH, ST], F32, tag="ph")
                for a in range(0, L, 512):
                    e = min(a + 512, L)
                    nc.tensor.matmul(pfoh[:, a:e], fohF, fo[:, a:e],
                                     start=True, stop=True)
                nc.vector.scalar_tensor_tensor(
                    outB[:, cs:cs + L], pfoh[:, :L], biasF, maskF[:, :L],
                    AOp.add, AOp.mult)

                # ---- vox offsets + voted coords ----
                pv = pbig.tile([3, ST], F32, tag="p")
                for a in range(0, L, 512):
                    e = min(a + 512, L)
                    nc.tensor.matmul(pv[:, a:e], w3, f2[:, a:e],
                                     start=True, stop=True)
                cvt = vpool.tile([3, ST], BF, tag="cvt")
                nc.sync.dma_start(cvt[:, :L], cvs_d[:, cs:cs + L])
                voff = vpool.tile([3, ST], BF, tag="voff")
                nc.scalar.activation(voff[:, :L], pv[:, :L],
                                     Act.Identity, bias=c3)
                nc.sync.dma_start(outV_d[:, cs:cs + L], voff[:, :L])
                vpre = vpool.tile([3, ST], BF, tag="vpre")
                nc.gpsimd.tensor_tensor(vpre[:, :L], voff[:, :L],
                                        cvt[:, :L], AOp.add)
                voted = vpool.tile([3, ST], BF, tag="voted")
                nc.vector.tensor_scalar(voted[:, :L], vpre[:, :L],
                                        mn3, mx3, AOp.max, AOp.min)
                nc.sync.dma_start(outW_d[:, cs:cs + L], voted[:, :L])

            for a, b in chunks:
                nc.sync.dma_start(outS_d[:, a:b], outS[:, a:b])
                nc.sync.dma_start(outB_d[:, a:b], outB[:, a:b])

    nc.finalize()
    return nc


def _host_prep(feats, coords_xyz, batch_idx,
               off_w1, off_g1, off_b1, off_w2, off_g2, off_b2, off_w3,
               fo_w, fo_g, fo_b, sem_w, sem_b, cen_w, cls_w, cls_b, reg_w,
               scales):
    f64 = np.float64
    N = feats.shape[0]

    # ---- neighbor lookup (identical to reference's sorted-key search) ----
    c1 = coords_xyz.astype(np.int64) + 1
    key = ((batch_idx.astype(np.int64) * HASH_D + c1[:, 0]) * HASH_D
           + c1[:, 1]) * HASH_D + c1[:, 2]
    order = np.argsort(key, kind="stable")
    skey = key[order]
    pos = np.searchsorted(skey, key)
    rep = order[pos]                      # first voxel with same key

    # ---- fused weights (BN folded; ELU+1 handled via bias shifts) ----
    W1 = off_w1.astype(f64) * off_g1.astype(f64)[None, :]
    b1 = off_b1.astype(f64)
    W2 = off_w2.astype(f64) * off_g2.astype(f64)[None, :]
    b2 = off_b2.astype(f64) - W2.sum(0)
    W3 = off_w3.astype(f64)
    c3 = -W3.sum(0)
    Wc = fo_w[13].astype(f64) * fo_g.astype(f64)[None, :]
    bc = fo_b.astype(f64)

    # ---- conv input: gather + fold rare non-center taps via Wc13^-1 ----
    G = feats.astype(f64)[rep]
    Winv = np.linalg.inv(fo_w[13].astype(f64))
    k = 0
    for dx in (-1, 0, 1):
        for dy in (-1, 0, 1):
            for dz in (-1, 0, 1):
                if (dx, dy, dz) != (0, 0, 0):
                    nk = key + (dx * HASH_D + dy) * HASH_D + dz
                    p = np.clip(np.searchsorted(skey, nk), 0, N - 1)
                    hit = skey[p] == nk
                    if hit.any():
                        dst = np.nonzero(hit)[0]
                        src = order[p[hit]]
                        A = fo_w[k].astype(f64) @ Winv
                        np.add.at(G, dst, feats.astype(f64)[src] @ A)
                k += 1

    # ---- j-major head layout: rows [cls 0:18 | cen 18 | regJ 19:127] ----
    sc64 = scales.astype(f64)
    semJF = np.zeros((C, FOH), f64)
    fohF = np.zeros((C, FOH), f64)
    thrF = np.zeros(FOH, np.float32)
    biasF = np.zeros(FOH, np.float32)
    semJF[:, 0:N_CLS] = sem_w.astype(f64)
    fohF[:, 0:N_CLS] = cls_w.astype(f64)
    thrF[0:N_CLS] = LOGIT_THR - sem_b.astype(f64)
    biasF[0:N_CLS] = cls_b.astype(f64) - cls_w.astype(f64).sum(0)
    # cen row: mask always on
    fohF[:, N_CLS] = cen_w.astype(f64)[:, 0]
    thrF[N_CLS] = -1e30
    biasF[N_CLS] = -cen_w.astype(f64).sum(0)[0]
    for j in range(N_REG):
        for c in range(N_CLS):
            r = 19 + j * N_CLS + c
            semJF[:, r] = sem_w.astype(f64)[:, c]
            fohF[:, r] = reg_w.astype(f64)[:, j] * sc64[c]
            thrF[r] = LOGIT_THR - sem_b.astype(f64)[c]
            biasF[r] = -reg_w.astype(f64)[:, j].sum() * sc64[c]

    # ---- weights blob ----
    wb = np.zeros((C, 1025), BF16)
    wb[:, 0:128] = W1.astype(BF16)
    wb[:, 128:256] = Wc.astype(BF16)
    wb[:, 256:384] = W2.astype(BF16)
    wb[:, 384:511] = semJF.astype(BF16)
    wb[:, 511:638] = fohF.astype(BF16)
    wb[:, 638:641] = W3.astype(BF16)
    wb[0, 641:769] = (b1 + 1.0).astype(BF16)
    wb[0, 769:897] = (bc + 1.0).astype(BF16)
    wb[0, 897:1025] = (b2 + 1.0).astype(BF16)

    # ---- f32 per-partition scalars ----
    mx = (coords_xyz.max(0) + 1).astype(f64) * VS
    mn = (coords_xyz.min(0) - 1).astype(f64) * VS
    sc = np.zeros((C, 7), np.float32)
    sc[:, 6] = -1.0
    sc[0:N_CLS, 0] = sem_b.astype(f64)
    sc[0:FOH, 1] = thrF
    sc[0:FOH, 2] = biasF
    sc[0:3, 3] = c3
    sc[0:3, 4] = mn
    sc[0:3, 5] = mx

    # ---- transposed, channel-major activations ----
    fT = np.ascontiguousarray(feats.T)
    gTf = np.ascontiguousarray(G.astype(np.float32).T)
    cT = coords_xyz.T.astype(np.float32) * VS

    wts = {"wb": wb, "sc": sc}
    in_maps = []
    for c in range(N_CORES):
        s, e = c * PER_CORE, (c + 1) * PER_CORE
        m = dict(wts)
        m["xT"] = np.ascontiguousarray(fT[:, s:e]).astype(BF16)
        m["gT"] = np.ascontiguousarray(gTf[:, s:e]).astype(BF16)
        m["cvs"] = np.ascontiguousarray(cT[:, s:e]).astype(BF16)
        in_maps.append(m)
    return in_maps


_CACHED = {}

# regJ row 19 + j*18 + c  ->  output col 43 + c*6 + j
_REG_COLS = np.empty(FOH - 19, np.int64)
for _j in range(N_REG):
    for _c in range(N_CLS):
        _REG_COLS[_j * N_CLS + _c] = 43 + _c * N_REG + _j


def _assemble(res, n):
    """Map device outputs to reference layout [n, 151] f32."""
    o = np.empty((n, OUT_ROWS), np.float32)
    o[:, 0:18] = res["outS"][:, :n].astype(np.float32).T
    o[:, 18:21] = res["outV"][:, :n].astype(np.float32).T
    o[:, 21:24] = res["outW"][:, :n].astype(np.float32).T
    outB = res["outB"][:, :n].astype(np.float32)
    o[:, 24] = outB[N_CLS]
    o[:, 25:43] = outB[0:N_CLS].T
    o[:, _REG_COLS] = outB[19:FOH].T
    return o


def kernel(**inputs):
    inputs = {k: np.asarray(v) for k, v in inputs.items()}
    in_maps = _host_prep(**inputs)
    if "nc" not in _CACHED:
        _CACHED["nc"] = _build_program()
    nc = _CACHED["nc"]
    res = run_bass_kernel_spmd(nc, in_maps, core_ids=list(range(N_CORES)))
    out = np.empty((N_VOX, OUT_ROWS), np.float32)
    for c in range(N_CORES):
        out[c * PER_CORE:(c + 1) * PER_CORE] = _assemble(
            res.results[c], PER_CORE)
    return out


# revision 7
# speedup vs baseline: 1.4577x; 1.3798x over previous
"""CAGroup3DHead kernel for 8 Trainium2 NeuronCores.

Strategy (data-parallel over voxels, per the sharding hint):
  - Host: integer index work (sorted-key neighbor lookup identical to the
    reference), weight fusion (BN folded into weights, ELU+1 bias shifts),
    and sharding marshaling (transpose to channel-major, bf16 cast,
    per-core slices).  The 3x3x3 sparse conv collapses to a gather: the
    (0,0,0) tap always hits, so conv_in = feats[rep]; the rare other-tap
    hits are folded into conv_in via W_k @ W_13^{-1} so the device conv is
    one dense matmul.
  - Semantic gate fast path: cls and reg_pc are gated by
    sigmoid(sem) > 0.15, i.e. sem_logit + sem_b > logit(0.15).  The host
    checks the rigorous bound max_i ||feats_i|| * max_c ||sem_w_c|| +
    sem_b_c < logit(0.15); when it holds (it does for the detection-prior
    bias -4.595 used here), every mask entry is exactly zero, so cls and
    reg_pc are exactly zero and the device skips them.  If the bound ever
    fails, a full device path (mask + masked heads) is built instead.
  - Device fast path (identical SPMD program on 8 cores): inputs are
    SBUF-resident (a few large DMAs); work proceeds in 1024-column
    supertiles.  Three 128x128 bf16 matmul chains (mlp1, conv, mlp2) with
    ELU+1 done as one scalar Exp + one relu + one min (exact); the three
    skinny heads (sem 18, voff 3, cen 1) share one [65,T] PSUM tile via
    32-row tile_position strips and drain with a single scalar op.
    Outputs are bf16 and stream out per-supertile; the host re-transposes
    and fills the zero sections during unsharding.
"""

import numpy as np
import ml_dtypes

import concourse.bass as bass
import concourse.bacc as bacc
import concourse.tile as tile
from concourse import mybir
from concourse.bass_utils import run_bass_kernel_spmd

BF16 = ml_dtypes.bfloat16

N_VOX = 100000
C = 128
N_CLS = 18
N_REG = 6
VS = 0.04
THR = 0.15
HASH_D = 260
N_CORES = 8
PER_CORE = N_VOX // N_CORES          # 12500
ST = 1024                            # supertile columns
N_ST = 13                            # 12 full + tail of 212
LOGIT_THR = float(np.log(THR / (1.0 - THR)))   # -1.734601..

FOH = 127                            # full-path head rows [cls|cen|regJ]
OUT_ROWS = 151

F32 = mybir.dt.float32
BF = mybir.dt.bfloat16
AOp = mybir.AluOpType
Act = mybir.ActivationFunctionType


def _build_fast():
    """Fast path: masks provably all-zero; compute sem/voff/voted/cen only."""
    nc = bacc.Bacc(trn_type="TRN2")

    xT_d = nc.dram_tensor("xT", [C, PER_CORE], BF, kind="ExternalInput")
    gT_d = nc.dram_tensor("gT", [C, PER_CORE], BF, kind="ExternalInput")
    cvs_d = nc.dram_tensor("cvs", [3, PER_CORE], BF, kind="ExternalInput")
    # bf16 weights: w1 0:128, wc 128:256, w2 256:384, sem 384:402,
    # w3 402:405, cen 405:406
    wb_d = nc.dram_tensor("wb", [C, 406], BF, kind="ExternalInput")
    # f32 per-partition scalars [128, 6]: col0 b1, col1 bc, col2 b2,
    # col3 bias65 (rows 0:18 semb, 32:35 c3, 64 cenb),
    # col4 mn (rows 32:35), col5 mx (rows 32:35)
    sc_d = nc.dram_tensor("sc", [C, 8], F32, kind="ExternalInput")
    outH_d = nc.dram_tensor("outH", [65, PER_CORE], BF, kind="ExternalOutput")
    outW_d = nc.dram_tensor("outW", [3, PER_CORE], BF, kind="ExternalOutput")

    chunks = [(0, 3072), (3072, 6144), (6144, 9216), (9216, PER_CORE)]

    with tile.TileContext(nc) as tc:
        with (
            tc.tile_pool(name="res", bufs=1) as res,
            tc.tile_pool(name="epool", bufs=4) as epool,
            tc.tile_pool(name="rpool", bufs=4) as rpool,
            tc.tile_pool(name="fpool", bufs=4) as fpool,
            tc.tile_pool(name="hpool", bufs=3) as hpool,
            tc.tile_pool(name="pbig", bufs=2, space=bass.MemorySpace.PSUM) as pbig,
            tc.tile_pool(name="phead", bufs=2, space=bass.MemorySpace.PSUM) as ph,
        ):
            wb = res.tile([C, 406], BF)
            sc = res.tile([C, 8], F32)
            nc.sync.dma_start(wb[:], wb_d[:])
            nc.sync.dma_start(sc[:], sc_d[:])
            w1 = wb[:, 0:128]
            wc = wb[:, 128:256]
            w2 = wb[:, 256:384]
            semw = wb[:, 384:402]
            w3 = wb[:, 402:405]
            cenw = wb[:, 405:406]
            b1 = sc[:, 0:1]
            bc = sc[:, 1:2]
            b2 = sc[:, 2:3]
            bias65 = sc[0:65, 3:4]
            mn3 = sc[32:35, 4:5]
            mx3 = sc[32:35, 5:6]

            xT = res.tile([C, PER_CORE], BF)
            gT = res.tile([C, PER_CORE], BF)
            cvsr = res.tile([35, PER_CORE], BF)
            for a, b in chunks:
                nc.sync.dma_start(xT[:, a:b], xT_d[:, a:b])
                nc.sync.dma_start(gT[:, a:b], gT_d[:, a:b])
                nc.sync.dma_start(cvsr[32:35, a:b], cvs_d[:, a:b])

            def elu_layer(w, src_sb, off, bias, L):
                p = pbig.tile([C, ST], F32, tag="p")
                for a in range(0, L, 512):
                    e = min(a + 512, L)
                    nc.tensor.matmul(p[:, a:e], w,
                                     src_sb[:, off + a:off + e],
                                     start=True, stop=True)
                et = epool.tile([C, ST], BF, tag="e")
                nc.scalar.activation(et[:, :L], p[:, :L], Act.Exp, bias=bias)
                rt = rpool.tile([C, ST], BF, tag="r")
                nc.vector.tensor_scalar(rt[:, :L], p[:, :L], bias, 0.0,
                                        AOp.add, AOp.max)
                ft = fpool.tile([C, ST], BF, tag="f")
                nc.vector.scalar_tensor_tensor(
                    ft[:, :L], rt[:, :L], 1.0, et[:, :L], AOp.add, AOp.min)
                return ft

            for k in range(N_ST):
                cs = k * ST
                L = min(ST, PER_CORE - cs)

                f1 = elu_layer(w1, xT, cs, b1, L)
                fo = elu_layer(wc, gT, cs, bc, L)
                f2 = elu_layer(w2, f1, 0, b2, L)

                # ---- heads: sem 0:18, voff 32:35, cen 64:65 in one psum ----
                phd = ph.tile([65, ST], F32, tag="ph")
                for a in range(0, L, 512):
                    e = min(a + 512, L)
                    nc.tensor.matmul(phd[0:18, a:e], semw,
                                     xT[:, cs + a:cs + e],
                                     start=True, stop=True,
                                     tile_position=(0, 0))
                    nc.tensor.matmul(phd[32:35, a:e], w3, f2[:, a:e],
                                     start=True, stop=True,
                                     tile_position=(0, 32))
                    nc.tensor.matmul(phd[64:65, a:e], cenw, fo[:, a:e],
                                     start=True, stop=True,
                                     tile_position=(0, 64))
                sv = hpool.tile([65, ST], BF, tag="sv")
                nc.scalar.activation(sv[:, :L], phd[:, :L], Act.Identity,
                                     bias=bias65)
                nc.sync.dma_start(outH_d[:, cs:cs + L], sv[:, :L])

                # ---- voted = clip(voff + coords*VS, mn, mx) ----
                vp = hpool.tile([35, ST], BF, tag="vp")
                nc.gpsimd.tensor_tensor(vp[32:35, :L], sv[32:35, :L],
                                        cvsr[32:35, cs:cs + L], AOp.add)
                vt = hpool.tile([35, ST], BF, tag="vt")
                nc.vector.tensor_scalar(vt[32:35, :L], vp[32:35, :L],
                                        mn3, mx3, AOp.max, AOp.min)
                nc.sync.dma_start(outW_d[:, cs:cs + L], vt[32:35, :L])

    nc.finalize()
    return nc


def _build_full():
    """Fallback: full mask + masked-heads path (used if the zero-mask
    bound fails).  j-major reg layout, one is_gt + one STT for heads."""
    nc = bacc.Bacc(trn_type="TRN2")

    xT_d = nc.dram_tensor("xT", [C, PER_CORE], BF, kind="ExternalInput")
    gT_d = nc.dram_tensor("gT", [C, PER_CORE], BF, kind="ExternalInput")
    cvs_d = nc.dram_tensor("cvs", [3, PER_CORE], BF, kind="ExternalInput")
    # w1 0:128, wc 128:256, w2 256:384, semJF 384:511, fohF 511:638,
    # w3 638:641, sem 641:659, cen 659:660
    wb_d = nc.dram_tensor("wb", [C, 660], BF, kind="ExternalInput")
    # col0 b2, col1 bias65, col2 mn, col3 mx, col4 thrF, col5 biasF
    sc_d = nc.dram_tensor("sc", [C, 8], F32, kind="ExternalInput")
    outH_d = nc.dram_tensor("outH", [65, PER_CORE], BF, kind="ExternalOutput")
    outW_d = nc.dram_tensor("outW", [3, PER_CORE], BF, kind="ExternalOutput")
    outB_d = nc.dram_tensor("outB", [FOH, PER_CORE], BF, kind="ExternalOutput")

    chunks = [(0, 3072), (3072, 6144), (6144, 9216), (9216, PER_CORE)]

    with tile.TileContext(nc) as tc:
        with (
            tc.tile_pool(name="res", bufs=1) as res,
            tc.tile_pool(name="epool", bufs=4) as epool,
            tc.tile_pool(name="rpool", bufs=4) as rpool,
            tc.tile_pool(name="fpool", bufs=4) as fpool,
            tc.tile_pool(name="hpool", bufs=3) as hpool,
            tc.tile_pool(name="mpool", bufs=2) as mpool,
            tc.tile_pool(name="pbig", bufs=2, space=bass.MemorySpace.PSUM) as pbig,
            tc.tile_pool(name="phead", bufs=1, space=bass.MemorySpace.PSUM) as ph,
            tc.tile_pool(name="pfoh", bufs=1, space=bass.MemorySpace.PSUM) as pf,
        ):
            wb = res.tile([C, 660], BF)
            sc = res.tile([C, 8], F32)
            nc.sync.dma_start(wb[:], wb_d[:])
            nc.sync.dma_start(sc[:], sc_d[:])
            w1 = wb[:, 0:128]
            wc = wb[:, 128:256]
            w2 = wb[:, 256:384]
            semJF = wb[:, 384:511]
            fohF = wb[:, 511:638]
            w3 = wb[:, 638:641]
            semw = wb[:, 641:659]
            cenw = wb[:, 659:660]
            b1 = sc[:, 0:1]
            bc = sc[:, 1:2]
            b2 = sc[:, 2:3]
            bias65 = sc[0:65, 3:4]
            mn3 = sc[32:35, 4:5]
            mx3 = sc[32:35, 5:6]
            thrF = sc[0:FOH, 6:7]
            biasF = sc[0:FOH, 7:8]

            xT = res.tile([C, PER_CORE], BF)
            gT = res.tile([C, PER_CORE], BF)
            cvsr = res.tile([35, PER_CORE], BF)
            for a, b in chunks:
                nc.sync.dma_start(xT[:, a:b], xT_d[:, a:b])
                nc.sync.dma_start(gT[:, a:b], gT_d[:, a:b])
                nc.sync.dma_start(cvsr[32:35, a:b], cvs_d[:, a:b])

            for k in range(N_ST):
                cs = k * ST
                L = min(ST, PER_CORE - cs)

                def elu_layer(w, src_sb, off, bias):
                    p = pbig.tile([C, ST], F32, tag="p")
                    for a in range(0, L, 512):
                        e = min(a + 512, L)
                        nc.tensor.matmul(p[:, a:e], w,
                                         src_sb[:, off + a:off + e],
                                         start=True, stop=True)
                    et = epool.tile([C, ST], BF, tag="e")
                    nc.scalar.activation(et[:, :L], p[:, :L], Act.Exp,
                                         bias=bias)
                    rt = rpool.tile([C, ST], BF, tag="r")
                    nc.vector.tensor_scalar(rt[:, :L], p[:, :L], bias,
                                            0.0, AOp.add, AOp.max)
                    ft = fpool.tile([C, ST], BF, tag="f")
                    nc.vector.scalar_tensor_tensor(
                        ft[:, :L], rt[:, :L], 1.0, et[:, :L],
                        AOp.add, AOp.min)
                    return ft

                f1 = elu_layer(w1, xT, cs, b1)
                fo = elu_layer(wc, gT, cs, bc)
                f2 = elu_layer(w2, f1, 0, b2)

                phd = ph.tile([65, ST], F32, tag="ph")
                for a in range(0, L, 512):
                    e = min(a + 512, L)
                    nc.tensor.matmul(phd[0:18, a:e], semw,
                                     xT[:, cs + a:cs + e],
                                     start=True, stop=True,
                                     tile_position=(0, 0))
                    nc.tensor.matmul(phd[32:35, a:e], w3, f2[:, a:e],
                                     start=True, stop=True,
                                     tile_position=(0, 32))
                    nc.tensor.matmul(phd[64:65, a:e], cenw, fo[:, a:e],
                                     start=True, stop=True,
                                     tile_position=(0, 64))
                sv = hpool.tile([65, ST], BF, tag="sv")
                nc.scalar.activation(sv[:, :L], phd[:, :L], Act.Identity,
                                     bias=bias65)
                nc.sync.dma_start(outH_d[:, cs:cs + L], sv[:, :L])

                vp = hpool.tile([35, ST], BF, tag="vp")
                nc.gpsimd.tensor_tensor(vp[32:35, :L], sv[32:35, :L],
                                        cvsr[32:35, cs:cs + L], AOp.add)
                vt = hpool.tile([35, ST], BF, tag="vt")
                nc.vector.tensor_scalar(vt[32:35, :L], vp[32:35, :L],
                                        mn3, mx3, AOp.max, AOp.min)
                nc.sync.dma_start(outW_d[:, cs:cs + L], vt[32:35, :L])

                # masked heads: (foh + biasF) * (semJ > thrF)
                psem = ph.tile([FOH, ST], F32, tag="ph")
                for a in range(0, L, 512):
                    e = min(a + 512, L)
                    nc.tensor.matmul(psem[:, a:e], semJF,
                                     xT[:, cs + a:cs + e],
                                     start=True, stop=True)
                maskF = mpool.tile([FOH, ST], BF, tag="maskF")
                nc.vector.tensor_scalar(maskF[:, :L], psem[:, :L], thrF,
                                        None, AOp.is_gt)
                pfo = pf.tile([FOH, ST], F32, tag="pfoh")
                for a in range(0, L, 512):
                    e = min(a + 512, L)
                    nc.tensor.matmul(pfo[:, a:e], fohF, fo[:, a:e],
                                     start=True, stop=True)
                hb = mpool.tile([FOH, ST], BF, tag="hb")
                nc.vector.scalar_tensor_tensor(
                    hb[:, :L], pfo[:, :L], biasF, maskF[:, :L],
                    AOp.add, AOp.mult)
                nc.sync.dma_start(outB_d[:, cs:cs + L], hb[:, :L])

    nc.finalize()
    return nc


def _host_prep(feats, coords_xyz, batch_idx,
               off_w1, off_g1, off_b1, off_w2, off_g2, off_b2, off_w3,
               fo_w, fo_g, fo_b, sem_w, sem_b, cen_w, cls_w, cls_b, reg_w,
               scales):
    f64 = np.float64
    N = feats.shape[0]

    # ---- zero-mask bound: |x.w_c| <= max||x|| * ||w_c|| (Cauchy-Schwarz)
    feats64 = feats.astype(f64)
    max_row = np.sqrt((feats64 * feats64).sum(1).max())
    colnrm = np.sqrt((sem_w.astype(f64) ** 2).sum(0))
    zero_mask = bool(
        np.all(max_row * colnrm + sem_b.astype(f64) < LOGIT_THR - 1e-3))

    # ---- neighbor lookup (identical to reference's sorted-key search) ----
    c1 = coords_xyz.astype(np.int64) + 1
    key = ((batch_idx.astype(np.int64) * HASH_D + c1[:, 0]) * HASH_D
           + c1[:, 1]) * HASH_D + c1[:, 2]
    order = np.argsort(key, kind="stable")
    skey = key[order]
    pos = np.searchsorted(skey, key)
    rep = order[pos]                      # first voxel with same key

    # ---- fused weights (BN folded; ELU+1 handled via bias shifts) ----
    W1 = off_w1.astype(f64) * off_g1.astype(f64)[None, :]
    b1 = off_b1.astype(f64)
    W2 = off_w2.astype(f64) * off_g2.astype(f64)[None, :]
    b2 = off_b2.astype(f64) - W2.sum(0)
    W3 = off_w3.astype(f64)
    c3 = -W3.sum(0)
    Wc = fo_w[13].astype(f64) * fo_g.astype(f64)[None, :]
    bc = fo_b.astype(f64)

    # ---- conv input: gather + fold rare non-center taps via Wc13^-1 ----
    G = feats64[rep]
    Winv = np.linalg.inv(fo_w[13].astype(f64))
    k = 0
    for dx in (-1, 0, 1):
        for dy in (-1, 0, 1):
            for dz in (-1, 0, 1):
                if (dx, dy, dz) != (0, 0, 0):
                    nk = key + (dx * HASH_D + dy) * HASH_D + dz
                    p = np.clip(np.searchsorted(skey, nk), 0, N - 1)
                    hit = skey[p] == nk
                    if hit.any():
                        dst = np.nonzero(hit)[0]
                        src = order[p[hit]]
                        A = fo_w[k].astype(f64) @ Winv
                        np.add.at(G, dst, feats64[src] @ A)
                k += 1

    # ---- shared scalar columns ----
    mx = (coords_xyz.max(0) + 1).astype(f64) * VS
    mn = (coords_xyz.min(0) - 1).astype(f64) * VS
    bias65 = np.zeros(65, f64)
    bias65[0:N_CLS] = sem_b.astype(f64)
    bias65[32:35] = c3
    bias65[64] = -cen_w.astype(f64).sum(0)[0]

    sc = np.zeros((C, 8), np.float32)
    sc[:, 0] = b1
    sc[:, 1] = bc
    sc[:, 2] = b2
    sc[0:65, 3] = bias65
    sc[32:35, 4] = mn
    sc[32:35, 5] = mx

    nwb = 406 if zero_mask else 660
    wb = np.zeros((C, nwb), BF16)
    wb[:, 0:128] = W1.astype(BF16)
    wb[:, 128:256] = Wc.astype(BF16)
    wb[:, 256:384] = W2.astype(BF16)
    if zero_mask:
        wb[:, 384:402] = sem_w.astype(f64).astype(BF16)
        wb[:, 402:405] = W3.astype(BF16)
        wb[:, 405:406] = cen_w.astype(f64).astype(BF16)
    else:
        # j-major layout: rows [cls 0:18 | cen 18 | regJ 19:127]
        sc64 = scales.astype(f64)
        semJF = np.zeros((C, FOH), f64)
        fohF = np.zeros((C, FOH), f64)
        thrF = np.zeros(FOH, np.float32)
        biasF = np.zeros(FOH, np.float32)
        semJF[:, 0:N_CLS] = sem_w.astype(f64)
        fohF[:, 0:N_CLS] = cls_w.astype(f64)
        thrF[0:N_CLS] = LOGIT_THR - sem_b.astype(f64)
        biasF[0:N_CLS] = cls_b.astype(f64) - cls_w.astype(f64).sum(0)
        fohF[:, N_CLS] = cen_w.astype(f64)[:, 0]
        thrF[N_CLS] = -1e30
        biasF[N_CLS] = -cen_w.astype(f64).sum(0)[0]
        for j in range(N_REG):
            for cc in range(N_CLS):
                r = 19 + j * N_CLS + cc
                semJF[:, r] = sem_w.astype(f64)[:, cc]
                fohF[:, r] = reg_w.astype(f64)[:, j] * sc64[cc]
                thrF[r] = LOGIT_THR - sem_b.astype(f64)[cc]
                biasF[r] = -reg_w.astype(f64)[:, j].sum() * sc64[cc]
        wb[:, 384:511] = semJF.astype(BF16)
        wb[:, 511:638] = fohF.astype(BF16)
        wb[:, 638:641] = W3.astype(BF16)
        wb[:, 641:659] = sem_w.astype(f64).astype(BF16)
        wb[:, 659:660] = cen_w.astype(f64).astype(BF16)
        sc[0:FOH, 6] = thrF
        sc[0:FOH, 7] = biasF

    # ---- transposed, channel-major activations ----
    fT = np.ascontiguousarray(feats.T)
    gTf = np.ascontiguousarray(G.astype(np.float32).T)
    cT = coords_xyz.T.astype(np.float32) * VS

    wts = {"wb": wb, "sc": sc}
    in_maps = []
    for c in range(N_CORES):
        s, e = c * PER_CORE, (c + 1) * PER_CORE
        m = dict(wts)
        m["xT"] = np.ascontiguousarray(fT[:, s:e]).astype(BF16)
        m["gT"] = np.ascontiguousarray(gTf[:, s:e]).astype(BF16)
        m["cvs"] = np.ascontiguousarray(cT[:, s:e]).astype(BF16)
        in_maps.append(m)
    return in_maps, zero_mask


_CACHED = {}

# full path: regJ row 19 + j*18 + c  ->  output col 43 + c*6 + j
_REG_COLS = np.empty(FOH - 19, np.int64)
for _j in range(N_REG):
    for _c in range(N_CLS):
        _REG_COLS[_j * N_CLS + _c] = 43 + _c * N_REG + _j


def _assemble(res, zero_mask, n):
    """Map device outputs to reference layout [n, 151] f32."""
    o = np.zeros((n, OUT_ROWS), np.float32)
    outH = res["outH"][:, :n].astype(np.float32)
    o[:, 0:18] = outH[0:18].T
    o[:, 18:21] = outH[32:35].T
    o[:, 21:24] = res["outW"][:, :n].astype(np.float32).T
    o[:, 24] = outH[64]
    if not zero_mask:
        outB = res["outB"][:, :n].astype(np.float32)
        o[:, 25:43] = outB[0:N_CLS].T
        o[:, _REG_COLS] = outB[19:FOH].T
    return o


def kernel(**inputs):
    inputs = {k: np.asarray(v) for k, v in inputs.items()}
    in_maps, zero_mask = _host_prep(**inputs)
    ckey = "fast" if zero_mask else "full"
    if ckey not in _CACHED:
        _CACHED[ckey] = _build_fast() if zero_mask else _build_full()
    nc = _CACHED[ckey]
    res = run_bass_kernel_spmd(nc, in_maps, core_ids=list(range(N_CORES)))
    out = np.empty((N_VOX, OUT_ROWS), np.float32)
    for c in range(N_CORES):
        out[c * PER_CORE:(c + 1) * PER_CORE] = _assemble(
            res.results[c], zero_mask, PER_CORE)
    return out
